# revision 22
# baseline (speedup 1.0000x reference)
"""Neural CDE forward pass on 8 Trainium2 NeuronCores (Bass/Tile).

Math (per batch element b):
    z0 = u0 @ Wi + bi                                   [64]
    for t in 0..164:
        h  = relu(z @ W1 + b1)                          [128]
        f  = tanh(h @ W2 + b2)                          [512] -> [64, 8]
        z += einsum('hi,i->h', f, dx_t)                 dx_t = coeffs[t+1]-coeffs[t]
    out_t = z_t @ Wr + br  for every t (166 values)

The scan is chaotic: perturbations amplify ~1e4x over the 165 steps, so
every matmul must run in exact fp32 (bf16/f32r weights give 10%+ final
error). fp32 matmuls stream at 1/4 rate on trn2 PE and their cost is
~4*N cycles, independent of K and M; the design therefore minimizes the
number of N=512 matmul slots per step.

Kernel design (per core, batch shard B=512 on the matmul free dim):
  - Split state: z = z_a + z_b, stacked as z_stack [128, B] fp32 in SBUF
    (rows 0..63 = z_a, 64..127 = z_b). The split lets the einsum-reduce
    matmuls run as 2 column-tiled PAIRS (concurrent 64-wide outputs)
    instead of 4 serial M=65 matmuls, and mm1 contracts the sum for free:
  - h:  h_ps = [W1;W1].T @ z_stack     (K=128, one matmul slot)
  - f:  fT = W2_j.T @ h, j=0..3        (4 slots into one 4-bank PSUM tile)
  - tanh per bank on ScalarE with fused per-partition bias b2_j.
  - einsum: g_j = f_j * dx_rep elementwise fp32 (VectorE + GpSimd),
    dx_rep[p, b] = dx[b, p % 8] (host pre-replicated, DMA streamed);
    e_ps [128, B]: rows 0..63  = S0.T g_0 + S2.T g_2   (col group 0-1)
                   rows 64..127= S1.T g_1 + S3.T g_3   (col group 2-3)
    via 2 slots of column-tiled matmul pairs, S_j [128, 64] selection
    matrices: S_j[p, 16j + p//8] = 1. Then z_stack += e_ps (one DVE add).
  - readout: wz = z_stack * [Wr;Wr] (per-partition scale, VectorE), then
    GpSimd partition_all_reduce sums all 128 partitions = z @ Wr; row 0
    is DMA'd to DRAM. br is added on the host.
"""

import numpy as np

IN_CH = 8
HID = 64
MLP_W = 128
OUT = 1
B_FULL, T = 4096, 166
NSTEP = T - 1
N_CORES = 8
B = B_FULL // N_CORES  # 512
NBANK = 4  # f feature banks of 128

# tuning knobs
RELU_ON = "act"  # "act" | "dve"
G_ON_GPSIMD = 1  # how many of the 4 g-multiplies run on GpSimd
NCHAIN = 4  # independent batch chains interleaved to hide dependency stalls
RED_LAG = 2  # einsum-reduce of chain c emitted RED_LAG chain-slots later

_CACHE = {}


def _build_bass():
    from contextlib import ExitStack

    import concourse.bass_isa as bass_isa
    import concourse.tile as tile
    from concourse import bacc, mybir

    f32 = mybir.dt.float32
    AF = mybir.ActivationFunctionType
    ALU = mybir.AluOpType

    nc = bacc.Bacc("TRN2", target_bir_lowering=False, debug=False)

    u0t = nc.dram_tensor("u0t", [IN_CH, B], f32, kind="ExternalInput")
    dxt = nc.dram_tensor("dxt", [NSTEP, 128, B], f32, kind="ExternalInput")
    w1s = nc.dram_tensor("w1s", [MLP_W, MLP_W], f32, kind="ExternalInput")
    b1 = nc.dram_tensor("b1", [MLP_W, 1], f32, kind="ExternalInput")
    w2 = nc.dram_tensor("w2", [MLP_W, NBANK, 128], f32, kind="ExternalInput")
    b2 = nc.dram_tensor("b2", [128, NBANK], f32, kind="ExternalInput")
    wi = nc.dram_tensor("wi", [IN_CH, MLP_W], f32, kind="ExternalInput")
    smat = nc.dram_tensor("smat", [128, NBANK, HID], f32, kind="ExternalInput")
    wrs = nc.dram_tensor("wrs", [MLP_W, 1], f32, kind="ExternalInput")
    outp = nc.dram_tensor("outp", [T, B], f32, kind="ExternalOutput")

    with tile.TileContext(nc) as tc, ExitStack() as ctx:
        const = ctx.enter_context(tc.tile_pool(name="const", bufs=1))
        zpool = ctx.enter_context(tc.tile_pool(name="zpool", bufs=2))
        hpool = ctx.enter_context(tc.tile_pool(name="hpool", bufs=2))
        fpool = ctx.enter_context(tc.tile_pool(name="fpool", bufs=2))
        gpool = ctx.enter_context(tc.tile_pool(name="gpool", bufs=3))
        wzpool = ctx.enter_context(tc.tile_pool(name="wzpool", bufs=2))
        dxpool = ctx.enter_context(tc.tile_pool(name="dxpool", bufs=4))
        psum_h = ctx.enter_context(tc.tile_pool(name="psum_h", bufs=2, space="PSUM"))
        psum_f = ctx.enter_context(tc.tile_pool(name="psum_f", bufs=3, space="PSUM"))
        psum_e = ctx.enter_context(tc.tile_pool(name="psum_e", bufs=3, space="PSUM"))

        w1s_sb = const.tile([MLP_W, MLP_W], f32)
        nc.sync.dma_start(w1s_sb[:], w1s[:])
        b1_sb = const.tile([MLP_W, 1], f32)
        nc.sync.dma_start(b1_sb[:], b1[:])
        w2_sb = const.tile([MLP_W, NBANK, 128], f32)
        nc.sync.dma_start(w2_sb[:], w2[:])
        b2_sb = const.tile([128, NBANK], f32)
        nc.sync.dma_start(b2_sb[:], b2[:])
        wi_sb = const.tile([IN_CH, MLP_W], f32)
        nc.sync.dma_start(wi_sb[:], wi[:])
        s_sb = const.tile([128, NBANK, HID], f32)
        nc.sync.dma_start(s_sb[:], smat[:])
        wrs_sb = const.tile([MLP_W, 1], f32)
        nc.sync.dma_start(wrs_sb[:], wrs[:])
        u0t_sb = const.tile([IN_CH, B], f32)
        nc.sync.dma_start(u0t_sb[:], u0t[:])

        Bc = B // NCHAIN  # batch per chain

        def readout(z_stack, row, c):
            cs = slice(c * Bc, (c + 1) * Bc)
            wz = wzpool.tile([MLP_W, Bc], f32, tag="wz")
            nc.vector.tensor_scalar_mul(wz[:], z_stack[:], wrs_sb[:, 0:1])
            red = wzpool.tile([MLP_W, Bc], f32, tag="red")
            nc.gpsimd.partition_all_reduce(
                red[:], wz[:], MLP_W, bass_isa.ReduceOp.add
            )
            nc.sync.dma_start(outp[row : row + 1, cs], red[0:1, :])

        # init: z_a = z0 (rows 0..63), z_b = 0 (rows 64..127)
        z_sb = []
        for c in range(NCHAIN):
            cs = slice(c * Bc, (c + 1) * Bc)
            z0_ps = psum_e.tile([MLP_W, Bc], f32, tag="e_ps", name=f"z0_ps{c}")
            nc.tensor.matmul(z0_ps[:], wi_sb[:], u0t_sb[:, cs], start=True, stop=True)
            z_c = zpool.tile([MLP_W, Bc], f32, tag=f"z{c}", name=f"z_sb{c}")
            nc.vector.tensor_copy(z_c[:], z0_ps[:])
            readout(z_c, 0, c)
            z_sb.append(z_c)

        # Software pipeline: the einsum-reduce of chain c is emitted RED_LAG
        # chain-slots after its mm2 burst, so the PE's in-order queue never
        # waits on the serial tanh -> g -> reduce -> z-add chain: other
        # chains' matmuls fill the gaps.
        dx_tiles = {}
        st = [dict(h=None, f=None, g=None) for _ in range(NCHAIN)]

        def frag_mm1(c, t):
            s = st[c]
            h_ps = psum_h.tile([MLP_W, Bc], f32, name="h_ps")
            nc.tensor.matmul(h_ps[:], w1s_sb[:], z_sb[c][:], start=True, stop=True)
            h_sb = hpool.tile([MLP_W, Bc], f32, name="h_sb")
            if RELU_ON == "act":
                nc.scalar.activation(h_sb[:], h_ps[:], AF.Relu, bias=b1_sb[:, 0:1])
            else:
                nc.vector.tensor_scalar(
                    h_sb[:], h_ps[:], b1_sb[:, 0:1], 0.0, ALU.add, ALU.max
                )
            s["h"] = h_sb

        def frag_mm2(c, t):
            s = st[c]
            f_ps = psum_f.tile([128, NBANK * Bc], f32, name="f_ps")
            for j in range(NBANK):
                nc.tensor.matmul(
                    f_ps[:, j * Bc : (j + 1) * Bc], w2_sb[:, j, :], s["h"][:],
                    start=True, stop=True,
                )
            f_sb = fpool.tile([128, NBANK * Bc], f32, name="f_sb")
            for j in range(NBANK):
                nc.scalar.activation(
                    f_sb[:, j * Bc : (j + 1) * Bc],
                    f_ps[:, j * Bc : (j + 1) * Bc],
                    AF.Tanh, bias=b2_sb[:, j : j + 1],
                )
            s["f"] = f_sb

        def frag_g(c, t):
            s = st[c]
            cs = slice(c * Bc, (c + 1) * Bc)
            dx_sb = dx_tiles[t]
            g_sb = gpool.tile([128, NBANK * Bc], f32, name="g_sb")
            for j in range(NBANK):
                eng = nc.gpsimd if j < G_ON_GPSIMD else nc.vector
                eng.tensor_mul(
                    g_sb[:, j * Bc : (j + 1) * Bc],
                    s["f"][:, j * Bc : (j + 1) * Bc],
                    dx_sb[:, cs],
                )
            s["g"] = g_sb

        def frag_red(c, t):
            s = st[c]
            e_ps = psum_e.tile([MLP_W, Bc], f32, tag="e_ps", name="e_ps")
            for j in range(NBANK):
                half = j % 2  # 0 -> rows 0..63, 1 -> rows 64..127
                nc.tensor.matmul(
                    e_ps[64 * half : 64 * half + 64, :],
                    s_sb[:, j, :],
                    s["g"][:, j * Bc : (j + 1) * Bc],
                    start=j < 2, stop=j >= 2,
                    # the sim's zero-region tracker ignores the partition
                    # offset, so the rows-64..127 pair falsely collides
                    # with the rows-0..63 pair
                    skip_group_check=half == 1,
                    tile_position=(0, 64 * half),
                )
            z_prev = z_sb[c]
            z_sb[c] = zpool.tile([MLP_W, Bc], f32, tag=f"z{c}", name=f"z_sb{c}")
            nc.vector.tensor_add(z_sb[c][:], e_ps[:], z_prev[:])
            readout(z_sb[c], t + 1, c)

        def dma_dx(t):
            dx_sb = dxpool.tile([128, B], f32, name="dx_sb")
            nc.sync.dma_start(dx_sb[:], dxt[t])
            dx_tiles[t] = dx_sb
            if t - 4 in dx_tiles:
                del dx_tiles[t - 4]

        def red_slot(c, t):
            """(chain, step) whose reduce is emitted in slot (c, t)."""
            cr = (c - RED_LAG) % NCHAIN
            tr = t if c >= RED_LAG else t - 1
            return cr, tr

        for t in range(NSTEP):
            dma_dx(t)
            for c in range(NCHAIN):
                frag_mm1(c, t)
                cr, tr = red_slot(c, t)
                if tr >= 0:
                    frag_red(cr, tr)
                frag_mm2(c, t)
                frag_g(c, t)
        # epilogue: reduces still in flight
        for c in range(RED_LAG):
            cr, tr = red_slot(c, NSTEP)
            frag_red(cr, tr)

    nc.compile()
    return nc


def _prep_host(u0, coeffs, W1, b1, W2, b2, Wi, bi, Wr, br):
    f32 = np.float32

    u0t_full = np.empty((IN_CH, B_FULL), f32)
    u0t_full[: IN_CH - 1] = u0.T
    u0t_full[IN_CH - 1] = 1.0

    dX = (coeffs[:, 1:] - coeffs[:, :-1]).astype(f32)  # [B_FULL, NSTEP, IN_CH]
    dxt_small = np.ascontiguousarray(dX.transpose(1, 2, 0))  # [NSTEP, 8, B_FULL]
    dxt_full = np.tile(dxt_small, (1, 128 // IN_CH, 1))

    # stacked mm1 weights: h = W1.T @ z_a + W1.T @ z_b
    w1s = np.zeros((MLP_W, MLP_W), f32)
    w1s[:HID] = W1
    w1s[HID:] = W1

    # init: z_a = z0, z_b = 0
    wi_mat = np.zeros((IN_CH, MLP_W), f32)
    wi_mat[: IN_CH - 1, :HID] = Wi
    wi_mat[IN_CH - 1, :HID] = bi

    w2_banks = np.ascontiguousarray(W2.reshape(MLP_W, NBANK, 128))
    b2_banks = np.ascontiguousarray(b2.reshape(NBANK, 128).T)

    p = np.arange(128)
    s_full = np.zeros((128, NBANK, HID), f32)
    for j in range(NBANK):
        s_full[p, j, 16 * j + p // IN_CH] = 1.0

    wr_stack = np.concatenate([Wr[:, 0], Wr[:, 0]]).reshape(MLP_W, 1).astype(f32)

    return {
        "u0t": u0t_full,
        "dxt": dxt_full,
        "w1s": w1s,
        "b1": np.ascontiguousarray(b1.astype(f32).reshape(MLP_W, 1)),
        "w2": w2_banks.astype(f32),
        "b2": b2_banks.astype(f32),
        "wi": wi_mat,
        "smat": s_full,
        "wrs": wr_stack,
    }


def _make_in_maps(full):
    in_maps = []
    for c in range(N_CORES):
        sl = slice(c * B, (c + 1) * B)
        in_maps.append(
            {
                "u0t": np.ascontiguousarray(full["u0t"][:, sl]),
                "dxt": np.ascontiguousarray(full["dxt"][:, :, sl]),
                "w1s": full["w1s"],
                "b1": full["b1"],
                "w2": full["w2"],
                "b2": full["b2"],
                "wi": full["wi"],
                "smat": full["smat"],
                "wrs": full["wrs"],
            }
        )
    return in_maps


def kernel(u0, coeffs, W1, b1, W2, b2, Wi, bi, Wr, br):
    from concourse.bass_utils import run_bass_kernel_spmd

    br = np.asarray(br, np.float32).reshape(OUT)
    full = _prep_host(
        np.asarray(u0, np.float32), np.asarray(coeffs, np.float32),
        np.asarray(W1, np.float32), np.asarray(b1, np.float32),
        np.asarray(W2, np.float32), np.asarray(b2, np.float32),
        np.asarray(Wi, np.float32), np.asarray(bi, np.float32),
        np.asarray(Wr, np.float32).reshape(HID, OUT), br,
    )
    in_maps = _make_in_maps(full)

    if "nc" not in _CACHE:
        _CACHE["nc"] = _build_bass()
    nc = _CACHE["nc"]

    res = run_bass_kernel_spmd(nc, in_maps, core_ids=list(range(N_CORES)))
    outs = res.results

    out_full = np.empty((B_FULL, T, OUT), np.float32)
    for c in range(N_CORES):
        out_full[c * B : (c + 1) * B, :, 0] = outs[c]["outp"].T
    out_full += br[0]
    return out_full


# revision 26
# speedup vs baseline: 5.3174x; 5.3174x over previous
"""Neural CDE forward pass on 8 Trainium2 NeuronCores (Bass/Tile).

Math (per batch element b):
    z0 = u0 @ Wi + bi                                   [64]
    for t in 0..164:
        h  = relu(z @ W1 + b1)                          [128]
        f  = tanh(h @ W2 + b2)                          [512] -> [64, 8]
        z += einsum('hi,i->h', f, dx_t)                 dx_t = coeffs[t+1]-coeffs[t]
    out_t = z_t @ Wr + br  for every t (166 values)

The scan is chaotic: perturbations amplify ~1e4x over the 165 steps, so
every matmul must run in exact fp32 (bf16/f32r weights give 10%+ final
error). fp32 matmuls stream at 1/4 rate on trn2 PE and cost ~4*N cycles
independent of K and M, so the design minimizes matmul count and keeps
the PE queue dense.

Kernel design (per core, batch shard B=512 split into NCHAIN independent
chains on the matmul free dim):
  - State zT [64+1, Bc] fp32 in SBUF per chain; row 64 carries the running
    readout out_t = z_t @ Wr + br.
  - h:  h_ps = W1.T @ zT            (K=64, one matmul)
  - f:  fT = W2_j.T @ h, j=0..3     (4 matmuls into one PSUM tile)
  - tanh per bank on ScalarE with fused per-partition bias b2_j.
  - einsum: g_j = f_j * dx_rep elementwise fp32 on VectorE,
    dx_rep[p, b] = dx[b, p % 8] (host pre-replicated, DMA streamed);
    e = sum_j S_j'.T @ g_j accumulated over 4 matmuls in PSUM with
    S_j' [128, 65]: S_j'[p, 16j + p//8] = 1 and column 64 = S_j @ Wr,
    which makes e[64] = Wr . e_z — the readout accumulates for free.
  - z_new = z_old + e (one VectorE add); row 64 is DMA'd per step.
  - Software pipeline: NCHAIN chains interleaved; the reduce of chain c
    is emitted RED_LAG chain-slots after its mm2 burst so the in-order
    PE queue never stalls on the serial tanh -> g -> reduce -> add chain.
"""

import numpy as np

IN_CH = 8
HID = 64
MLP_W = 128
OUT = 1
B_FULL, T = 4096, 166
NSTEP = T - 1
N_CORES = 8
B = B_FULL // N_CORES  # 512
NBANK = 4  # f feature banks of 128

# tuning knobs
RELU_ON = "act"  # "act" | "dve"
G_ON_GPSIMD = 0  # how many of the 4 g-multiplies run on GpSimd
NCHAIN = 4  # independent batch chains interleaved to hide dependency stalls
RED_LAG = 2  # einsum-reduce of chain c emitted RED_LAG chain-slots later
REPEAT = 1  # run the whole scan REPEAT times (timing amplification only)

_CACHE = {}


def _build_bass():
    from contextlib import ExitStack

    import concourse.tile as tile
    from concourse import bacc, mybir

    f32 = mybir.dt.float32
    AF = mybir.ActivationFunctionType
    ALU = mybir.AluOpType

    nc = bacc.Bacc("TRN2", target_bir_lowering=False, debug=False)

    u0t = nc.dram_tensor("u0t", [IN_CH, B], f32, kind="ExternalInput")
    dxt = nc.dram_tensor("dxt", [NSTEP, 128, B], f32, kind="ExternalInput")
    w1 = nc.dram_tensor("w1", [HID, MLP_W], f32, kind="ExternalInput")
    b1 = nc.dram_tensor("b1", [MLP_W, 1], f32, kind="ExternalInput")
    w2 = nc.dram_tensor("w2", [MLP_W, NBANK, 128], f32, kind="ExternalInput")
    b2 = nc.dram_tensor("b2", [128, NBANK], f32, kind="ExternalInput")
    wi = nc.dram_tensor("wi", [IN_CH, HID + 1], f32, kind="ExternalInput")
    smat = nc.dram_tensor("smat", [128, NBANK, HID + 1], f32, kind="ExternalInput")
    outp = nc.dram_tensor("outp", [T, B], f32, kind="ExternalOutput")

    Bc = B // NCHAIN

    with tile.TileContext(nc) as tc, ExitStack() as ctx:
        const = ctx.enter_context(tc.tile_pool(name="const", bufs=1))
        zpool = ctx.enter_context(tc.tile_pool(name="zpool", bufs=2))
        hpool = ctx.enter_context(tc.tile_pool(name="hpool", bufs=2))
        fpool = ctx.enter_context(tc.tile_pool(name="fpool", bufs=2))
        gpool = ctx.enter_context(tc.tile_pool(name="gpool", bufs=3))
        dxpool = ctx.enter_context(tc.tile_pool(name="dxpool", bufs=4))
        psum_h = ctx.enter_context(tc.tile_pool(name="psum_h", bufs=2, space="PSUM"))
        psum_f = ctx.enter_context(tc.tile_pool(name="psum_f", bufs=3, space="PSUM"))
        psum_e = ctx.enter_context(tc.tile_pool(name="psum_e", bufs=3, space="PSUM"))

        w1_sb = const.tile([HID, MLP_W], f32)
        nc.sync.dma_start(w1_sb[:], w1[:])
        b1_sb = const.tile([MLP_W, 1], f32)
        nc.sync.dma_start(b1_sb[:], b1[:])
        w2_sb = const.tile([MLP_W, NBANK, 128], f32)
        nc.sync.dma_start(w2_sb[:], w2[:])
        b2_sb = const.tile([128, NBANK], f32)
        nc.sync.dma_start(b2_sb[:], b2[:])
        wi_sb = const.tile([IN_CH, HID + 1], f32)
        nc.sync.dma_start(wi_sb[:], wi[:])
        s_sb = const.tile([128, NBANK, HID + 1], f32)
        nc.sync.dma_start(s_sb[:], smat[:])
        u0t_sb = const.tile([IN_CH, B], f32)
        nc.sync.dma_start(u0t_sb[:], u0t[:])

        z_sb = [None] * NCHAIN
        dx_tiles = {}
        st = [dict(h=None, f=None, g=None) for _ in range(NCHAIN)]

        def init_chains():
            # init per chain: rows 0..63 = z0, row 64 = out_0
            for c in range(NCHAIN):
                cs = slice(c * Bc, (c + 1) * Bc)
                z0_ps = psum_e.tile(
                    [HID + 1, Bc], f32, tag="e_ps", name=f"z0_ps{c}"
                )
                nc.tensor.matmul(
                    z0_ps[:], wi_sb[:], u0t_sb[:, cs], start=True, stop=True
                )
                z_c = zpool.tile([HID + 1, Bc], f32, tag=f"z{c}", name=f"z_sb{c}")
                nc.vector.tensor_copy(z_c[:], z0_ps[:])
                nc.sync.dma_start(outp[0:1, cs], z_c[HID : HID + 1, :])
                z_sb[c] = z_c

        def frag_mm1(c, t):
            s = st[c]
            h_ps = psum_h.tile([MLP_W, Bc], f32, name="h_ps")
            nc.tensor.matmul(
                h_ps[:], w1_sb[:], z_sb[c][0:HID, :], start=True, stop=True
            )
            h_sb = hpool.tile([MLP_W, Bc], f32, name="h_sb")
            if RELU_ON == "act":
                nc.scalar.activation(h_sb[:], h_ps[:], AF.Relu, bias=b1_sb[:, 0:1])
            else:
                nc.vector.tensor_scalar(
                    h_sb[:], h_ps[:], b1_sb[:, 0:1], 0.0, ALU.add, ALU.max
                )
            s["h"] = h_sb

        def frag_mm2(c, t):
            s = st[c]
            f_ps = psum_f.tile([128, NBANK * Bc], f32, name="f_ps")
            for j in range(NBANK):
                nc.tensor.matmul(
                    f_ps[:, j * Bc : (j + 1) * Bc], w2_sb[:, j, :], s["h"][:],
                    start=True, stop=True,
                )
            f_sb = fpool.tile([128, NBANK * Bc], f32, name="f_sb")
            for j in range(NBANK):
                nc.scalar.activation(
                    f_sb[:, j * Bc : (j + 1) * Bc],
                    f_ps[:, j * Bc : (j + 1) * Bc],
                    AF.Tanh, bias=b2_sb[:, j : j + 1],
                )
            s["f"] = f_sb

        def frag_g(c, t):
            s = st[c]
            cs = slice(c * Bc, (c + 1) * Bc)
            dx_sb = dx_tiles[t]
            g_sb = gpool.tile([128, NBANK * Bc], f32, name="g_sb")
            for j in range(NBANK):
                eng = nc.gpsimd if j < G_ON_GPSIMD else nc.vector
                eng.tensor_mul(
                    g_sb[:, j * Bc : (j + 1) * Bc],
                    s["f"][:, j * Bc : (j + 1) * Bc],
                    dx_sb[:, cs],
                )
            s["g"] = g_sb

        def frag_red(c, t):
            s = st[c]
            cs = slice(c * Bc, (c + 1) * Bc)
            e_ps = psum_e.tile([HID + 1, Bc], f32, tag="e_ps", name="e_ps")
            for j in range(NBANK):
                nc.tensor.matmul(
                    e_ps[:], s_sb[:, j, :], s["g"][:, j * Bc : (j + 1) * Bc],
                    start=j == 0, stop=j == NBANK - 1,
                )
            z_prev = z_sb[c]
            z_sb[c] = zpool.tile([HID + 1, Bc], f32, tag=f"z{c}", name=f"z_sb{c}")
            nc.vector.tensor_add(z_sb[c][:], e_ps[:], z_prev[:])
            nc.sync.dma_start(outp[t + 1 : t + 2, cs], z_sb[c][HID : HID + 1, :])

        def dma_dx(t):
            dx_sb = dxpool.tile([128, B], f32, name="dx_sb")
            nc.sync.dma_start(dx_sb[:], dxt[t])
            dx_tiles[t] = dx_sb
            if t - 4 in dx_tiles:
                del dx_tiles[t - 4]

        def red_slot(c, t):
            """(chain, step) whose reduce is emitted in slot (c, t)."""
            cr = (c - RED_LAG) % NCHAIN
            tr = t if c >= RED_LAG else t - 1
            return cr, tr

        for _rep in range(REPEAT):
            init_chains()
            dx_tiles.clear()
            for t in range(NSTEP):
                dma_dx(t)
                for c in range(NCHAIN):
                    frag_mm1(c, t)
                    cr, tr = red_slot(c, t)
                    if tr >= 0:
                        frag_red(cr, tr)
                    frag_mm2(c, t)
                    frag_g(c, t)
            # epilogue: reduces still in flight
            for c in range(RED_LAG):
                cr, tr = red_slot(c, NSTEP)
                frag_red(cr, tr)

    nc.compile()
    return nc


def _prep_host(u0, coeffs, W1, b1, W2, b2, Wi, bi, Wr, br):
    f32 = np.float32

    u0t_full = np.empty((IN_CH, B_FULL), f32)
    u0t_full[: IN_CH - 1] = u0.T
    u0t_full[IN_CH - 1] = 1.0

    dX = (coeffs[:, 1:] - coeffs[:, :-1]).astype(f32)  # [B_FULL, NSTEP, IN_CH]
    dxt_small = np.ascontiguousarray(dX.transpose(1, 2, 0))  # [NSTEP, 8, B_FULL]
    dxt_full = np.tile(dxt_small, (1, 128 // IN_CH, 1))

    wi_mat = np.empty((IN_CH, HID + 1), f32)
    wi_mat[: IN_CH - 1, :HID] = Wi
    wi_mat[IN_CH - 1, :HID] = bi
    wi_mat[: IN_CH - 1, HID] = (Wi @ Wr)[:, 0]
    wi_mat[IN_CH - 1, HID] = float(bi @ Wr[:, 0] + br[0])

    w2_banks = np.ascontiguousarray(W2.reshape(MLP_W, NBANK, 128))
    b2_banks = np.ascontiguousarray(b2.reshape(NBANK, 128).T)

    p = np.arange(128)
    s_full = np.zeros((128, NBANK, HID + 1), f32)
    for j in range(NBANK):
        s_full[p, j, 16 * j + p // IN_CH] = 1.0
        s_full[p, j, HID] = Wr[16 * j + p // IN_CH, 0]

    return {
        "u0t": u0t_full,
        "dxt": dxt_full,
        "w1": np.ascontiguousarray(W1.astype(f32)),
        "b1": np.ascontiguousarray(b1.astype(f32).reshape(MLP_W, 1)),
        "w2": w2_banks.astype(f32),
        "b2": b2_banks.astype(f32),
        "wi": wi_mat,
        "smat": s_full,
    }


def _make_in_maps(full):
    in_maps = []
    for c in range(N_CORES):
        sl = slice(c * B, (c + 1) * B)
        in_maps.append(
            {
                "u0t": np.ascontiguousarray(full["u0t"][:, sl]),
                "dxt": np.ascontiguousarray(full["dxt"][:, :, sl]),
                "w1": full["w1"],
                "b1": full["b1"],
                "w2": full["w2"],
                "b2": full["b2"],
                "wi": full["wi"],
                "smat": full["smat"],
            }
        )
    return in_maps


def kernel(u0, coeffs, W1, b1, W2, b2, Wi, bi, Wr, br):
    from concourse.bass_utils import run_bass_kernel_spmd

    full = _prep_host(
        np.asarray(u0, np.float32), np.asarray(coeffs, np.float32),
        np.asarray(W1, np.float32), np.asarray(b1, np.float32),
        np.asarray(W2, np.float32), np.asarray(b2, np.float32),
        np.asarray(Wi, np.float32), np.asarray(bi, np.float32),
        np.asarray(Wr, np.float32).reshape(HID, OUT),
        np.asarray(br, np.float32).reshape(OUT),
    )
    in_maps = _make_in_maps(full)

    if "nc" not in _CACHE:
        _CACHE["nc"] = _build_bass()
    nc = _CACHE["nc"]

    res = run_bass_kernel_spmd(nc, in_maps, core_ids=list(range(N_CORES)))
    outs = res.results

    out_full = np.empty((B_FULL, T, OUT), np.float32)
    for c in range(N_CORES):
        out_full[c * B : (c + 1) * B, :, 0] = outs[c]["outp"].T
    return out_full
